# revision 29
# baseline (speedup 1.0000x reference)
"""Causal MHA + out-projection Trainium2 kernel (B=64, S=512, E=256, H=4).

Sharding: data-parallel over batch, 8 batches per NeuronCore x 8 cores.

Per (batch, head) the attention is computed transpose-free:
  - S^T[k, q] = sum_d K[k,d] Q[q,d]  (chunks of 128 keys; q free dim,
    causally restricted to q >= 128*j for key-chunk j)
  - causal mask inside the diagonal 128x128 block is added via one extra
    matmul with constant triangular factors A,B such that
    (A^T B)[k,q] = -1e10 * (k - q) for q < k, else 0.
  - P^T = exp((S^T + mask)/8) on the scalar engine (one op per head)
  - O^T_aug = [V_h | ones64]^T @ P^T accumulated over key chunks:
    rows 0:64 = O^T (unnormalized), rows 64:128 = softmax denominator
    replicated 64x.
  - X^T rows for the head = O^T / denom  (one DVE divide, psum->sbuf)
  - Y[q,:] = X^T.T @ W^T + b  (q lands on partitions -> contiguous store)

Host-side prep: qry/key are pre-transposed to [B, E, S] and w_out to
W^T so that every device DMA is contiguous.
"""

import sys

if "/opt/trn_rl_repo" not in sys.path:
    sys.path.insert(0, "/opt/trn_rl_repo")

import numpy as np

import concourse.bass as bass
import concourse.mybir as mybir
import concourse.tile as tile
from concourse import bacc
from concourse.bass_utils import run_bass_kernel_spmd

B, S, E, H = 64, 512, 256, 4
D = E // H  # 64
P = 128
NCORES = 8
BPC = B // NCORES  # 8
NEG = -1.0e10

FP = mybir.dt.float32
FPR = mybir.dt.float32r
BF = mybir.dt.bfloat16

# S^T chunk -> column offset inside the 3-bank (1536 col) psum tile.
# chunk j covers keys [128j, 128j+128), q in [128j, 512) => width 512-128j.
# Packing: c0 fills bank 0, c1+c3 fill bank 1 exactly, c2 in bank 2 =>
# columns [0:1280) are contiguous with no uninitialized gap.
CHUNK_COL = [0, 512, 1024, 896]
ST_COLS = 1280  # exp() span

# matmul operand dtype: bf16 runs the PE at 1 cycle/row and halves input
# DMA; accumulation stays fp32 in PSUM.  "f32" would be exact but 4x slower.
MM_DT = "bf16"
TD = BF if MM_DT == "bf16" else FP
NP_TD = None  # numpy dtype for host-side cast, set below


def _mm(ap):
    return ap


def attention_kernel(ctx, tc, out, qT, kT, v, wT, bo):
    nc = tc.nc
    AF = mybir.ActivationFunctionType
    OP = mybir.AluOpType

    consts = ctx.enter_context(tc.tile_pool(name="consts", bufs=1))
    qk_pool = ctx.enter_context(tc.tile_pool(name="qk", bufs=2))
    v_pool = ctx.enter_context(tc.tile_pool(name="v", bufs=2))
    pt_pool = ctx.enter_context(tc.tile_pool(name="pt", bufs=3))
    xt_pool = ctx.enter_context(tc.tile_pool(name="xt", bufs=2))
    yo_pool = ctx.enter_context(tc.tile_pool(name="yo", bufs=2))
    dsb_pool = ctx.enter_context(tc.tile_pool(name="dsb", bufs=3))
    st_psum = ctx.enter_context(tc.tile_pool(name="st", bufs=2, space="PSUM"))
    # ot ([128,512]) and y ([128,2,256]) share one 1-bank slot tag so the
    # PV->normalize chain gets 2 buffers without exceeding 8 psum banks.
    oy_psum = ctx.enter_context(tc.tile_pool(name="oy", bufs=2, space="PSUM"))

    # ---- constants ----
    # W^T as out-proj rhs: [c-part, c-chunk, e]
    wt_sb = consts.tile([P, 2, E], TD)
    nc.sync.dma_start(wt_sb[:], wT.rearrange("(c p) e -> p c e", p=P))

    # bias via K=1 matmul: ones row (lhsT) x bias row (rhs)
    bias_f = consts.tile([1, E], FP, tag="bias_f")
    nc.sync.dma_start(bias_f[:], bo)
    brow = consts.tile([1, E], BF)
    nc.vector.tensor_copy(brow[:], bias_f[:])
    ones_f = consts.tile([1, P], FP, tag="ones_f")
    nc.gpsimd.memset(ones_f[:], 1.0)
    ones1 = consts.tile([1, P], BF)
    nc.vector.tensor_copy(ones1[:], ones_f[:])

    # causal-mask factors (bf16): (A^T B)[k,q] = NEG*(k-q) if q<k else 0
    mf = consts.tile([P, P], FP, tag="mf")
    a_bf = consts.tile([P, P], BF)
    b_bf = consts.tile([P, P], BF)
    # A[j,k] = 1 iff k > j   (keep iff -j + k - 1 >= 0)
    nc.gpsimd.memset(mf[:], 1.0)
    nc.gpsimd.affine_select(
        out=mf[:], in_=mf[:], compare_op=OP.is_ge, fill=0.0,
        base=-1, pattern=[[1, P]], channel_multiplier=-1,
    )
    nc.vector.tensor_copy(a_bf[:], mf[:])
    # B[j,q] = NEG iff q <= j else 0
    mf2 = consts.tile([P, P], FP, tag="mf2")
    nc.gpsimd.memset(mf2[:], NEG)
    nc.gpsimd.affine_select(
        out=mf2[:], in_=mf2[:], compare_op=OP.is_ge, fill=0.0,
        base=0, pattern=[[-1, P]], channel_multiplier=1,
    )
    nc.vector.tensor_copy(b_bf[:], mf2[:])

    # ---- software-pipelined emission over (batch, head-pair) units ----
    # Emitting S^T(p+1) before the exp/PV/normalize tail of p keeps the
    # in-order PE queue free of head-of-line stalls (PE always has the next
    # pair's QK matmuls ready while ACT/DVE drain the previous pair).
    pair_states = {}

    def emit_head(pu):
        b, hc = pu
        if hc == 0:
            qt_sb = qk_pool.tile([P, 2, S], TD, tag="qt", name=f"qt{b}")
            nc.sync.dma_start(qt_sb[:], qT[b].rearrange("(c p) s -> p c s", p=P))
            kt_sb = qk_pool.tile([P, 2, S], TD, tag="kt", name=f"kt{b}")
            nc.sync.dma_start(kt_sb[:], kT[b].rearrange("(c p) s -> p c s", p=P))
            va = v_pool.tile([P, 4, H, P], TD, tag="va", name=f"va{b}")
            nc.gpsimd.memset(va[:, :, :, D:], 1.0)
            for j in range(4):
                nc.sync.dma_start(
                    va[:, j, :, :D],
                    v[b, P * j : P * (j + 1)].rearrange("p (h d) -> p h d", d=D),
                )
            xt_sb = xt_pool.tile([P, 2, S], TD, tag="xt", name=f"xt{b}")
            pair_states[b] = (qt_sb, kt_sb, va, xt_sb)
        qt_sb, kt_sb, va, xt_sb = pair_states[b]

        st0 = st_psum.tile([P, ST_COLS], FP, tag="st", name=f"st0_{b}_{hc}")
        st1 = st_psum.tile([P, ST_COLS], FP, tag="st", name=f"st1_{b}_{hc}")
        st = [st0, st1]
        for j in range(4):
            qoff = P * j
            qr = S - qoff
            col = CHUNK_COL[j]
            for u in range(2):
                hp = D * u
                nc.tensor.matmul(
                    st[u][:, col : col + qr],
                    lhsT=kt_sb[hp : hp + D, hc, qoff : qoff + P],
                    rhs=qt_sb[hp : hp + D, hc, qoff:S],
                    start=True,
                    stop=False,
                )
            # additive causal mask closes each chunk's accumulation group
            # before the next chunk reuses the same psum bank
            for u in range(2):
                nc.tensor.matmul(
                    st[u][:, col : col + P],
                    lhsT=a_bf[:],
                    rhs=b_bf[:],
                    start=False,
                    stop=True,
                )
        return (b, hc, st, va, xt_sb)

    def emit_tail(state):
        b, hc, st, va, xt_sb = state
        for u in range(2):
            h = 2 * hc + u
            hp = D * u
            pt = pt_pool.tile([P, ST_COLS], TD, tag="pt", name=f"pt{b}_{h}")
            nc.scalar.activation(pt[:], st[u][:, 0:ST_COLS], AF.Exp, scale=0.125)

            ot = oy_psum.tile([P, S], FP, tag="oy", name=f"ot{b}_{h}")
            for j in range(4):
                qoff = P * j
                qr = S - qoff
                col = CHUNK_COL[j]
                nc.tensor.matmul(
                    ot[:, qoff:S],
                    lhsT=va[:, j, h, :],
                    rhs=pt[:, col : col + qr],
                    start=(j == 0),
                    stop=(j == 3),
                )

            # normalize: rows 0:64 O^T, rows 64:128 denom replicated 64x.
            # Cross-partition copy realigns the denom, then reciprocal +
            # aligned multiply write this head's X^T rows.
            d_sb = dsb_pool.tile([D, S], FP, tag="dsb", name=f"d{b}_{h}")
            nc.vector.tensor_copy(d_sb[:], ot[D:P, :])
            r_sb = dsb_pool.tile([D, S], FP, tag="rsb", name=f"r{b}_{h}")
            nc.vector.reciprocal_approx_fast(r_sb[:], d_sb[:])
            nc.vector.tensor_tensor(
                xt_sb[hp : hp + D, hc, :], ot[0:D, :], r_sb[:], OP.mult
            )

        if hc == 1:
            # out-projection; bias folded in as a K=1 matmul, egress on ACT
            for qp in range(2):
                y = oy_psum.tile([P, 2, E], FP, tag="oy", name=f"y{b}_{qp}")
                for qi in range(2):
                    qt_idx = 2 * qp + qi
                    for c in range(2):
                        nc.tensor.matmul(
                            y[:, qi, :],
                            lhsT=xt_sb[:, c, P * qt_idx : P * (qt_idx + 1)],
                            rhs=wt_sb[:, c, :],
                            start=(c == 0),
                            stop=False,
                        )
                    nc.tensor.matmul(
                        y[:, qi, :],
                        lhsT=ones1[:],
                        rhs=brow[:],
                        start=False,
                        stop=True,
                    )
                yout = yo_pool.tile([P, 2, E], FP, tag="yout", name=f"yo{b}_{qp}")
                nc.scalar.copy(yout[:], y[:])
                nc.sync.dma_start(
                    out[b].rearrange("(g p) e -> p g e", p=P)[
                        :, 2 * qp : 2 * qp + 2, :
                    ],
                    yout[:],
                )
            del pair_states[b]

    pairs = [(b, hc) for b in range(BPC) for hc in range(2)]
    pending = None
    for pu in pairs:
        state = emit_head(pu)
        if pending is not None:
            emit_tail(pending)
        pending = state
    emit_tail(pending)

def build_nc(bpc=BPC):
    from contextlib import ExitStack

    nc = bacc.Bacc("TRN2", target_bir_lowering=False, debug=False)
    qT = nc.dram_tensor("qT", [bpc, E, S], TD, kind="ExternalInput").ap()
    kT = nc.dram_tensor("kT", [bpc, E, S], TD, kind="ExternalInput").ap()
    v = nc.dram_tensor("v", [bpc, S, E], TD, kind="ExternalInput").ap()
    wT = nc.dram_tensor("wT", [E, E], TD, kind="ExternalInput").ap()
    bo = nc.dram_tensor("bo", [1, E], FP, kind="ExternalInput").ap()
    out = nc.dram_tensor("out", [bpc, S, E], FP, kind="ExternalOutput").ap()

    with tile.TileContext(nc) as tc:
        with ExitStack() as ctx:
            global BPC_ACTIVE
            # allow reduced-batch builds for simulation
            saved = globals()["BPC"]
            globals()["BPC"] = bpc
            try:
                attention_kernel(ctx, tc, out, qT, kT, v, wT, bo)
            finally:
                globals()["BPC"] = saved
    nc.compile()
    return nc


def _np_td():
    if MM_DT == "bf16":
        import ml_dtypes

        return np.dtype(ml_dtypes.bfloat16)
    return np.dtype(np.float32)


def make_in_maps(qry, key, val, w_out, b_out):
    td = _np_td()
    qT_all = np.ascontiguousarray(qry.transpose(0, 2, 1)).astype(td)
    kT_all = np.ascontiguousarray(key.transpose(0, 2, 1)).astype(td)
    val = val.astype(td)
    wT = np.ascontiguousarray(w_out.T).astype(td)
    bo = np.ascontiguousarray(b_out.reshape(1, E))
    maps = []
    for c in range(NCORES):
        sl = slice(c * BPC, (c + 1) * BPC)
        maps.append(
            {
                "qT": qT_all[sl],
                "kT": kT_all[sl],
                "v": np.ascontiguousarray(val[sl]),
                "wT": wT,
                "bo": bo,
            }
        )
    return maps


_NC_CACHE = {}


def _get_nc():
    if "nc" not in _NC_CACHE:
        _NC_CACHE["nc"] = build_nc()
    return _NC_CACHE["nc"]


def kernel(qry, key, val, w_out, b_out, **run_kwargs):
    nc = _get_nc()
    in_maps = make_in_maps(
        np.asarray(qry, dtype=np.float32),
        np.asarray(key, dtype=np.float32),
        np.asarray(val, dtype=np.float32),
        np.asarray(w_out, dtype=np.float32),
        np.asarray(b_out, dtype=np.float32),
    )
    res = run_bass_kernel_spmd(nc, in_maps, core_ids=list(range(NCORES)), **run_kwargs)
    out = np.concatenate([res.results[c]["out"] for c in range(NCORES)], axis=0)
    if run_kwargs:
        kernel.last_results = res
    return out
